# revision 14
# baseline (speedup 1.0000x reference)
"""AttDGCNN forward on 8 TRN2 NeuronCores — pure data parallel (1 cloud/core).

Per-core pipeline (one point cloud, N=1024, K=30):
- kNN: negated squared distances via fp32r matmuls; candidate index OR-packed
  into the fp32 distance mantissa low 10 bits; top-32 per point via segmented
  max8 + match_replace rounds on DVE (slots 30/31 overwritten with the self
  edge so exactly the top-30 neighbor set contributes).
- Gather: one-hot matmuls on the TensorEngine (no usable indexed DMA in this
  runtime).  One-hot [cand, edge] tiles are built by tensor_scalar is_equal
  (DVE 4x mode) from an fp16 index row replicated by tiny matmuls.  Layers
  1-3 gather B = x @ w1b directly (associativity); layer 4 gathers x.
- Edge MLP layers as bf16 matmuls, fp32 PSUM; max-aggregation over the 32
  edge slots via DVE tensor_reduce straight out of PSUM.
- SE block, spatial attention (conv1d k=7 as banded constant matmuls in
  point-major layout), residual projection, global pooling and the MLP head
  all on device.
Edge enumeration is k-major within each 128-point row tile: e = k*128 + n.
"""
import sys
import numpy as np

sys.path.insert(0, "/opt/trn_rl_repo")

K = 30
KP = 32          # padded K slots
N = 1024         # points per cloud
NSEG = 16        # segments for first-stage max8
SEG = N // NSEG
NT = 8           # row tiles of 128 points
EC = 512         # edges per e-chunk = 4 k-slices x 128 points
KS_PER_CHUNK = EC // 128  # 4
CHANNELS = [(3, 64), (64, 64), (64, 128), (128, 256)]
NUM_CLASSES = 49
D512 = 512

_CACHE = {}


# --------------------------------------------------------------------------
# Host-side parameter packing
# --------------------------------------------------------------------------
def _prep_host_inputs(params):
    import ml_dtypes
    bf16 = ml_dtypes.bfloat16
    aux = {}

    def chunk_rows(name, arr):
        for k in range(0, arr.shape[0], 128):
            aux[f"{name}_{k // 128}"] = np.ascontiguousarray(arr[k:k + 128])

    def col_pack(name, vec):
        # [dim] -> [128, ceil(dim/128)], column c = vec[c*128:(c+1)*128]
        dim = vec.shape[0]
        ncol = (dim + 127) // 128
        out = np.zeros((128, ncol), np.float32)
        for c in range(ncol):
            w = min(128, dim - c * 128)
            out[:w, c] = vec[c * 128:c * 128 + w]
        aux[name] = out

    for li, (cin, cout) in enumerate(CHANNELS):
        p = {k: np.asarray(v) for k, v in params[f"ec{li}"].items()}
        w1 = p["w1"].astype(np.float32)
        aux[f"w1a_{li}"] = np.ascontiguousarray((w1[:cin] - w1[cin:]).astype(bf16))
        aux[f"w1b_{li}"] = np.ascontiguousarray(w1[cin:].astype(bf16))
        col_pack(f"b1_{li}", p["b1"].astype(np.float32))
        chunk_rows(f"w2_{li}", p["w2"].astype(bf16))
        col_pack(f"b2_{li}", p["b2"].astype(np.float32))
        chunk_rows(f"w3_{li}", p["w3"].astype(bf16))
        col_pack(f"b3_{li}", p["b3"].astype(np.float32))
        chunk_rows(f"sew1_{li}", p["se_w1"].astype(np.float32))
        aux[f"sew2_{li}"] = p["se_w2"].astype(np.float32)
        if "res_w" in p:
            aux[f"resw_{li}"] = p["res_w"].astype(np.float32)
            aux[f"resb_{li}"] = p["res_b"].astype(np.float32).reshape(1, cout)
        sp = p["sp_conv"].astype(np.float32)   # [7, 2, 1] WIO
        for ch in range(2):
            wts = sp[:, ch, 0]
            mid = np.zeros((128, 128), np.float32)
            lo = np.zeros((128, 128), np.float32)
            hi = np.zeros((128, 128), np.float32)
            for t in range(7):
                off = t - 3
                for n_ in range(128):
                    m = n_ + off
                    if 0 <= m < 128:
                        mid[m, n_] += wts[t]
                    elif m < 0:
                        lo[128 + m, n_] += wts[t]
                    else:
                        hi[m - 128, n_] += wts[t]
            scl = (1.0 / cout) if ch == 0 else 1.0
            aux[f"band{ch}m_{li}"] = mid * scl
            aux[f"band{ch}l_{li}"] = lo * scl
            aux[f"band{ch}h_{li}"] = hi * scl

    qkv_w = np.asarray(params["qkv_w"]).astype(np.float32)
    chunk_rows("hw_v", qkv_w[:, 2 * D512:].copy())
    chunk_rows("hw_proj", np.asarray(params["proj_w"]).astype(np.float32))
    col_pack("hb_proj", np.asarray(params["proj_b"]).astype(np.float32))
    chunk_rows("hw_pf1", np.asarray(params["pf_w1"]).astype(np.float32))
    col_pack("hb_pf1", np.asarray(params["pf_b1"]).astype(np.float32))
    chunk_rows("hw_pf2", np.asarray(params["pf_w2"]).astype(np.float32))
    col_pack("hb_pf2", np.asarray(params["pf_b2"]).astype(np.float32))
    chunk_rows("hw_o1", np.asarray(params["out_w1"]).astype(np.float32))
    col_pack("hb_o1", np.asarray(params["out_b1"]).astype(np.float32))
    chunk_rows("hw_o2", np.asarray(params["out_w2"]).astype(np.float32))
    col_pack("hb_o2", np.asarray(params["out_b2"]).astype(np.float32))
    chunk_rows("hw_o3", np.asarray(params["out_w3"]).astype(np.float32))
    col_pack("hb_o3", np.asarray(params["out_b3"]).astype(np.float32))

    aux["iota_u32"] = np.broadcast_to(
        np.arange(N, dtype=np.uint32)[None, :], (128, N)).copy()
    aux["piota"] = (np.arange(128, dtype=np.float32)[:, None]
                    + 128.0 * np.arange(8, dtype=np.float32)[None, :]).copy()
    # tiled identity [128, EC] bf16 (xi-term selector: e % 128 == q)
    aux["i128rep"] = np.tile(np.eye(128, dtype=np.float32),
                             (1, KS_PER_CHUNK)).astype(bf16)
    aux["ident_f32"] = np.eye(128, dtype=np.float32)
    aux["ident_bf"] = np.eye(128, dtype=bf16)
    # k-slice selectors: sel[q, kk*128 + p] = (q == kk), fp16 [32, KP*128]
    sel = np.zeros((KP, KP * 128), np.float32)
    for kk in range(KP):
        sel[kk, kk * 128:(kk + 1) * 128] = 1.0
    aux["sel"] = sel.astype(np.float16)
    return aux


# --------------------------------------------------------------------------
# Device kernel builder
# --------------------------------------------------------------------------
def _build(nc, aux_shapes, debug=False):
    import concourse.bass as bass
    import concourse.mybir as mybir
    import concourse.tile as tile

    F32 = mybir.dt.float32
    F32R = mybir.dt.float32r
    I32 = mybir.dt.int32
    U32 = mybir.dt.uint32
    BF16 = mybir.dt.bfloat16
    FP16 = mybir.dt.float16
    Alu = mybir.AluOpType
    Act = mybir.ActivationFunctionType
    AX = mybir.AxisListType

    dram = {}
    dram["data"] = nc.dram_tensor("data", [3, N], F32, kind="ExternalInput").ap()
    for name, (shape, dtype) in aux_shapes.items():
        dt = {"float32": F32, "bfloat16": BF16, "float16": FP16,
              "uint32": U32}[dtype]
        dram[name] = nc.dram_tensor(name, list(shape), dt, kind="ExternalInput").ap()
    out_d = nc.dram_tensor("out", [NUM_CLASSES, 1], F32, kind="ExternalOutput").ap()
    dbg = {}
    if debug:
        for li, (_, cout) in enumerate(CHANNELS):
            nh = 2 if cout > 128 else 1
            chw = cout // nh
            for hh in range(nh):
                dbg[f"dx{li}_{hh}"] = nc.dram_tensor(
                    f"dx{li}_{hh}", [chw, N], F32, kind="ExternalOutput").ap()
            dbg[f"dh{li}"] = nc.dram_tensor(
                f"dh{li}", [chw, N], F32, kind="ExternalOutput").ap()
        dbg["didx0"] = nc.dram_tensor("didx0", [128, KP], F32,
                                      kind="ExternalOutput").ap()
        dbg["dcomp0"] = nc.dram_tensor("dcomp0", [128, N], F32,
                                       kind="ExternalOutput").ap()

    with tile.TileContext(nc) as tc:
        with tc.tile_pool(name="consts", bufs=1) as cpool, \
             tc.tile_pool(name="layers", bufs=1) as lpool, \
             tc.tile_pool(name="work", bufs=2) as wpool, \
             tc.tile_pool(name="ps", bufs=8, space="PSUM") as pspool:

            def psum(shape, tag="ps"):
                return pspool.tile(shape, F32, tag=tag, name=f"ps{nc.next_id()}")

            W = {}
            for name in aux_shapes:
                if name.startswith("hw_") or name.startswith("hb_"):
                    continue  # head weights stay in DRAM, loaded on demand
                ap = dram[name]
                t = cpool.tile(list(ap.shape), ap.dtype, tag=name, name=name + "_t")
                nc.sync.dma_start(t[:], ap[:])
                W[name] = t
            identf = W["ident_f32"]

            ones128 = cpool.tile([1, 128], F32R, name="ones128")
            nc.vector.memset(ones128[:].bitcast(F32), 1.0)
            ones512 = cpool.tile([1, 512], F32R, name="ones512")
            nc.vector.memset(ones512[:].bitcast(F32), 1.0)
            ones128h = cpool.tile([1, 128], FP16, name="ones128h")
            nc.vector.memset(ones128h[:], 1.0)
            onescol = cpool.tile([128, 1], F32, name="onescol")
            nc.vector.memset(onescol[:], 1.0)
            ones128f = cpool.tile([1, 128], F32, name="ones128f")
            nc.vector.memset(ones128f[:], 1.0)

            # ---- layer state ----
            x_fm = lpool.tile([3, N], F32, tag="x_fm0", name="x_fm_l0")
            nc.sync.dma_start(x_fm[:], dram["data"][:])
            x_fmr = lpool.tile([3, N], F32R, tag="x_fmr0", name="x_fmr_l0")
            nc.scalar.activation(x_fmr[:], x_fm[:], Act.Copy)
            xbf_fm = lpool.tile([3, N], BF16, tag="xbf_fm0", name="xbf_fm_l0")
            nc.scalar.activation(xbf_fm[:], x_fm[:], Act.Copy)
            xT = None
            feats = []   # list of (tiles, widths) per layer

            for li, (cin, cout) in enumerate(CHANNELS):
                assert cin <= 128
                nhalf = 2 if cout > 128 else 1
                ch = cout // nhalf
                use_bgather = (li < 3)

                # ---- squared norms (negated) ----
                xsq = wpool.tile([cin, N], F32, tag="xsq", bufs=1, name=f"xsq{li}")
                nc.scalar.square(xsq[:], x_fm[:])
                sqn = wpool.tile([1, N], F32R, tag="sqn", name=f"sqn{li}")
                for half in range(2):
                    sl = slice(half * 512, (half + 1) * 512)
                    sps = psum([1, 512])
                    nc.tensor.matmul(sps[:], onescol[:cin, :],
                                     xsq[:, sl], start=True, stop=True)
                    nc.scalar.activation(sqn[:, sl], sps[:], Act.Copy, scale=-1.0)
                x2 = wpool.tile([cin, N], F32R, tag="x2", bufs=1, name=f"x2_{li}")
                nc.scalar.activation(x2[:], x_fm[:], Act.Copy, scale=2.0)

                # ---- AT (and BT) point-major tiles [128, cout] bf16 ----
                at_tiles, bt_tiles = [], []
                for t in range(NT):
                    sl = slice(t * 128, (t + 1) * 128)
                    ps = psum([128, cout])
                    nc.tensor.matmul(ps[:], xbf_fm[:, sl], W[f"w1a_{li}"][:],
                                     start=True, stop=True)
                    at = lpool.tile([128, cout], BF16, tag=f"at_{t}", name=f"at{li}_{t}")
                    nc.scalar.activation(at[:], ps[:], Act.Copy)
                    at_tiles.append(at)
                    if use_bgather:
                        ps2 = psum([128, cout])
                        nc.tensor.matmul(ps2[:], xbf_fm[:, sl], W[f"w1b_{li}"][:],
                                         start=True, stop=True)
                        bt = lpool.tile([128, cout], BF16, tag=f"bt_{t}",
                                        name=f"bt{li}_{t}")
                        nc.scalar.activation(bt[:], ps2[:], Act.Copy)
                        bt_tiles.append(bt)

                h_out = [lpool.tile([ch, N], F32, tag=f"hout_{hh}",
                                    name=f"hout{li}_{hh}") for hh in range(nhalf)]

                # ---- per row tile ----
                for t in range(NT):
                    nsl = slice(t * 128, (t + 1) * 128)
                    comp = wpool.tile([128, N], F32, tag="comp", bufs=1, name=f"comp{li}_{t}")
                    for half in range(2):
                        msl = slice(half * 512, (half + 1) * 512)
                        dps = psum([128, 512])
                        nc.tensor.matmul(dps[:], x2[:, nsl],
                                         x_fmr[:, msl],
                                         start=True, stop=False)
                        nc.tensor.matmul(dps[:], sqn[:, nsl],
                                         ones512[:],
                                         start=False, stop=False)
                        nc.tensor.matmul(dps[:], ones128[:],
                                         sqn[:, msl],
                                         start=False, stop=True)
                        # clear low 10 mantissa bits, then OR the index in
                        nc.vector.tensor_scalar(comp[:, msl].bitcast(U32),
                                                dps[:].bitcast(U32),
                                                10, 10,
                                                op0=Alu.logical_shift_right,
                                                op1=Alu.logical_shift_left)
                        nc.vector.tensor_tensor(comp[:, msl].bitcast(U32),
                                                comp[:, msl].bitcast(U32),
                                                W["iota_u32"][:, msl],
                                                op=Alu.bitwise_or)
                    if debug and li == 0 and t == 0:
                        nc.sync.dma_start(dbg["dcomp0"][:], comp[:])
                    surv = wpool.tile([128, NSEG * 8], F32, tag="surv",
                                      name=f"surv{li}_{t}")
                    for s in range(NSEG):
                        nc.vector.max(surv[:, s * 8:(s + 1) * 8],
                                      comp[:, s * SEG:(s + 1) * SEG])
                    ext = wpool.tile([128, KP], F32, tag="ext", name=f"ext{li}_{t}")
                    for r in range(4):
                        nc.vector.max(ext[:, r * 8:(r + 1) * 8], surv[:])
                        if r < 3:
                            nc.vector.match_replace(
                                surv[:], in_to_replace=ext[:, r * 8:(r + 1) * 8],
                                in_values=surv[:], imm_value=-3.0e38)
                    idxu = wpool.tile([128, KP], U32, tag="idxu", name=f"idxu{li}_{t}")
                    nc.vector.tensor_scalar(idxu[:], ext[:].bitcast(U32),
                                            1023, None, op0=Alu.bitwise_and)
                    nc.vector.tensor_copy(idxu[:, 30:31], idxu[:, 0:1])
                    nc.vector.tensor_copy(idxu[:, 31:32], idxu[:, 0:1])
                    idxf = wpool.tile([128, KP], F32, tag="idxf", name=f"idxf{li}_{t}")
                    nc.vector.tensor_copy(idxf[:], idxu[:].bitcast(I32))
                    if debug and li == 0 and t == 0:
                        nc.sync.dma_start(dbg["didx0"][:], idxf[:])
                    tps = psum([KP, 128])
                    nc.tensor.transpose(tps[:], idxf[:], identf[:])
                    idxT = wpool.tile([KP, 128], FP16, tag="idxT", name=f"idxT{li}_{t}")
                    nc.scalar.activation(idxT[:], tps[:], Act.Copy)

                    # ---- e-chunks: 4 k-slices x 128 points each ----
                    for ecn in range(8):
                        rep_ps = psum([128, EC])
                        for j in range(KS_PER_CHUNK):
                            kk = ecn * KS_PER_CHUNK + j
                            nc.tensor.matmul(rep_ps[:, j * 128:(j + 1) * 128],
                                             W["sel"][:, kk * 128:(kk + 1) * 128],
                                             idxT[:, :],
                                             start=True, stop=True)
                        idxrep = wpool.tile([128, EC], FP16, tag="idxrep",
                                            name=f"ir{li}_{t}_{ecn}")
                        nc.scalar.activation(idxrep[:], rep_ps[:], Act.Copy)

                        hps = [psum([ch, EC]) for _ in range(nhalf)]
                        xg_ps = None if use_bgather else psum([cin, EC])
                        for h8 in range(8):
                            oh = wpool.tile([128, EC], BF16, tag=f"oh{h8 % 4}",
                                            name=f"oh{li}_{t}_{ecn}_{h8}")
                            nc.vector.tensor_scalar(oh[:], idxrep[:],
                                                    W["piota"][:, h8:h8 + 1], None,
                                                    op0=Alu.is_equal)
                            if use_bgather:
                                for hh in range(nhalf):
                                    csl = slice(hh * ch, (hh + 1) * ch)
                                    nc.tensor.matmul(hps[hh][:],
                                                     bt_tiles[h8][:, csl], oh[:],
                                                     start=(h8 == 0), stop=False)
                            else:
                                nc.tensor.matmul(xg_ps[:], xT[h8][:, :cin], oh[:],
                                                 start=(h8 == 0), stop=(h8 == 7))
                        if not use_bgather:
                            xg = wpool.tile([cin, EC], BF16, tag="xg",
                                            name=f"xg{li}_{t}_{ecn}")
                            nc.scalar.activation(xg[:], xg_ps[:], Act.Copy)
                            for hh in range(nhalf):
                                csl = slice(hh * ch, (hh + 1) * ch)
                                nc.tensor.matmul(hps[hh][:], W[f"w1b_{li}"][:, csl],
                                                 xg[:], start=True, stop=False)
                        for hh in range(nhalf):
                            csl = slice(hh * ch, (hh + 1) * ch)
                            nc.tensor.matmul(hps[hh][:], at_tiles[t][:, csl],
                                             W["i128rep"][:], start=False, stop=True)
                        h1 = [wpool.tile([ch, EC], BF16, tag=f"h1_{hh}",
                                         name=f"h1_{li}_{t}_{ecn}_{hh}")
                              for hh in range(nhalf)]
                        for hh in range(nhalf):
                            csl = slice(hh * ch, (hh + 1) * ch)
                            nc.scalar.activation(h1[hh][:], hps[hh][:], Act.Relu,
                                                 bias=W[f"b1_{li}"][:ch, hh:hh + 1])
                        h2ps = [psum([ch, EC]) for _ in range(nhalf)]
                        for hh in range(nhalf):
                            csl = slice(hh * ch, (hh + 1) * ch)
                            for kc in range(nhalf):
                                ksl = slice(kc * ch, (kc + 1) * ch)
                                nc.tensor.matmul(h2ps[hh][:],
                                                 W[f"w2_{li}_{kc}"][:ch, csl],
                                                 h1[kc][:],
                                                 start=(kc == 0),
                                                 stop=(kc == nhalf - 1))
                        h2 = [wpool.tile([ch, EC], BF16, tag=f"h2_{hh}",
                                         name=f"h2_{li}_{t}_{ecn}_{hh}")
                              for hh in range(nhalf)]
                        for hh in range(nhalf):
                            csl = slice(hh * ch, (hh + 1) * ch)
                            nc.scalar.activation(h2[hh][:], h2ps[hh][:], Act.Relu,
                                                 bias=W[f"b2_{li}"][:ch, hh:hh + 1])
                        h3ps = [psum([ch, EC]) for _ in range(nhalf)]
                        for hh in range(nhalf):
                            csl = slice(hh * ch, (hh + 1) * ch)
                            for kc in range(nhalf):
                                ksl = slice(kc * ch, (kc + 1) * ch)
                                nc.tensor.matmul(h3ps[hh][:],
                                                 W[f"w3_{li}_{kc}"][:ch, csl],
                                                 h2[kc][:],
                                                 start=(kc == 0),
                                                 stop=(kc == nhalf - 1))
                        # aggregate: max over the 4 k-slices, then max-accum
                        for hh in range(nhalf):
                            red = wpool.tile([ch, 128], F32, tag=f"red{hh}",
                                             name=f"red{li}_{t}_{ecn}_{hh}")
                            src = h3ps[hh][:].rearrange("p (k n) -> p n k",
                                                        k=KS_PER_CHUNK)
                            nc.vector.tensor_reduce(red[:], src, axis=AX.X,
                                                    op=Alu.max)
                            if ecn == 0:
                                nc.vector.tensor_copy(h_out[hh][:, nsl], red[:])
                            else:
                                nc.vector.tensor_tensor(h_out[hh][:, nsl],
                                                        h_out[hh][:, nsl], red[:],
                                                        op=Alu.max)

                if debug:
                    nc.sync.dma_start(dbg[f"dh{li}"][:], h_out[0][:])
                # ---- SE block ----
                c16 = cout // 16
                svec = wpool.tile([128, nhalf], F32, tag="svec", name=f"svec{li}")
                for hh in range(nhalf):
                    csl = slice(hh * ch, (hh + 1) * ch)
                    sm = wpool.tile([ch, 1], F32, tag="s_mean", name=f"sm{li}_{hh}")
                    nc.vector.tensor_reduce(sm[:], h_out[hh][:], axis=AX.X, op=Alu.add)
                    nc.vector.tensor_scalar(svec[:ch, hh:hh + 1], sm[:],
                                            1.0 / N, W[f"b3_{li}"][:ch, hh:hh + 1],
                                            op0=Alu.mult, op1=Alu.add)
                se1ps = psum([c16, 1])
                for hh in range(nhalf):
                    nc.tensor.matmul(se1ps[:],
                                     W[f"sew1_{li}_{hh}"][:ch, :],
                                     svec[:ch, hh:hh + 1],
                                     start=(hh == 0), stop=(hh == nhalf - 1))
                se1 = wpool.tile([c16, 1], F32, tag="se1", name=f"se1_{li}")
                nc.scalar.activation(se1[:], se1ps[:], Act.Relu)
                sig = wpool.tile([128, nhalf], F32, tag="sig", name=f"sig{li}")
                for hh in range(nhalf):
                    csl = slice(hh * ch, (hh + 1) * ch)
                    sps2 = psum([ch, 1])
                    nc.tensor.matmul(sps2[:], W[f"sew2_{li}"][:, csl],
                                     se1[:], start=True, stop=True)
                    nc.scalar.activation(sig[:ch, hh:hh + 1], sps2[:], Act.Sigmoid)
                hse = [wpool.tile([ch, N], F32, tag=f"hse{hh}", bufs=1,
                                  name=f"hse{li}_{hh}")
                       for hh in range(nhalf)]
                for hh in range(nhalf):
                    csl = slice(hh * ch, (hh + 1) * ch)
                    nc.vector.tensor_scalar(hse[hh][:], h_out[hh][:],
                                            W[f"b3_{li}"][:ch, hh:hh + 1],
                                            sig[:ch, hh:hh + 1],
                                            op0=Alu.add, op1=Alu.mult)

                # ---- spatial attention (point-major) + residual ----
                ymean = wpool.tile([128, NT], F32, tag="ymean", name=f"ym{li}")
                ymax = wpool.tile([128, NT], F32, tag="ymax", name=f"yx{li}")
                hseT = []
                for t in range(NT):
                    nsl = slice(t * 128, (t + 1) * 128)
                    ht = lpool.tile([128, cout], F32, tag=f"hseT_{t}",
                                    name=f"hseT{li}_{t}")
                    for hh in range(nhalf):
                        tp = psum([128, ch])
                        nc.tensor.transpose(tp[:], hse[hh][:, nsl], identf[:ch, :ch])
                        nc.scalar.activation(ht[:, hh * ch:(hh + 1) * ch], tp[:],
                                             Act.Copy)
                    hseT.append(ht)
                    nc.vector.tensor_reduce(ymean[:, t:t + 1], ht[:], axis=AX.X,
                                            op=Alu.add)
                    nc.vector.tensor_reduce(ymax[:, t:t + 1], ht[:], axis=AX.X,
                                            op=Alu.max)
                sigsp = wpool.tile([128, NT], F32, tag="sigsp", name=f"sp{li}")
                for t in range(NT):
                    terms = []
                    for (bandp, yt, ti) in ((f"band0m_{li}", ymean, t),
                                            (f"band1m_{li}", ymax, t),
                                            (f"band0l_{li}", ymean, t - 1),
                                            (f"band1l_{li}", ymax, t - 1),
                                            (f"band0h_{li}", ymean, t + 1),
                                            (f"band1h_{li}", ymax, t + 1)):
                        if 0 <= ti < NT:
                            terms.append((bandp, yt, ti))
                    cps = psum([128, 1])
                    for i, (bandp, yt, ti) in enumerate(terms):
                        nc.tensor.matmul(cps[:], W[bandp][:],
                                         yt[:, ti:ti + 1],
                                         start=(i == 0), stop=(i == len(terms) - 1))
                    nc.scalar.activation(sigsp[:, t:t + 1], cps[:], Act.Sigmoid)
                has_res = (cin != cout)
                xnT = []
                for t in range(NT):
                    nsl = slice(t * 128, (t + 1) * 128)
                    rps = psum([128, cout])
                    if has_res:
                        nc.tensor.matmul(rps[:], x_fm[:, nsl],
                                         W[f"resw_{li}"][:],
                                         start=True, stop=False)
                        nc.tensor.matmul(rps[:], ones128f[:],
                                         W[f"resb_{li}"][:],
                                         start=False, stop=True)
                    else:
                        nc.tensor.transpose(rps[:, :cin], x_fm[:, nsl],
                                            identf[:cin, :cin])
                    xt_new = lpool.tile([128, cout], F32, tag=f"xnT_{t}",
                                        name=f"xnT{li}_{t}")
                    tmp = wpool.tile([128, cout], F32, tag="xnT_tmp",
                                     name=f"xnTt{li}_{t}")
                    nc.vector.tensor_scalar(tmp[:], hseT[t][:],
                                            sigsp[:, t:t + 1], None, op0=Alu.mult)
                    nc.vector.tensor_tensor(xt_new[:], tmp[:], rps[:, :cout],
                                            op=Alu.add)
                    xnT.append(xt_new)

                # ---- next-layer x tensors (half tiles of <=128 rows) ----
                xf_tiles = [lpool.tile([ch, N], F32, tag=f"x_fm{li + 1}_{hh}",
                                       name=f"x_fm_l{li + 1}_{hh}")
                            for hh in range(nhalf)]
                for t in range(NT):
                    nsl = slice(t * 128, (t + 1) * 128)
                    for hh in range(nhalf):
                        csl = slice(hh * ch, (hh + 1) * ch)
                        tp = psum([ch, 128])
                        nc.tensor.transpose(tp[:], xnT[t][:, csl], identf[:])
                        nc.scalar.activation(xf_tiles[hh][:, nsl], tp[:], Act.Copy)
                feats.append((xf_tiles, ch))
                if debug:
                    for hh in range(nhalf):
                        nc.sync.dma_start(dbg[f"dx{li}_{hh}"][:], xf_tiles[hh][:])
                if li < 3:
                    assert nhalf == 1
                    x_fm = xf_tiles[0]
                    x_fmr = lpool.tile([cout, N], F32R, tag=f"x_fmr{li + 1}",
                                       name=f"x_fmr_l{li + 1}")
                    nc.scalar.activation(x_fmr[:], x_fm[:], Act.Copy)
                    xbf_fm = lpool.tile([cout, N], BF16, tag=f"xbf_fm{li + 1}",
                                        name=f"xbf_fm_l{li + 1}")
                    nc.scalar.activation(xbf_fm[:], x_fm[:], Act.Copy)
                if li == 2:
                    xT = []
                    for t in range(NT):
                        xb = lpool.tile([128, cout], BF16, tag=f"xT_{t}",
                                        name=f"xTt{li}_{t}")
                        nc.scalar.activation(xb[:], xnT[t][:], Act.Copy)
                        xT.append(xb)

            # ---- global pooling ----
            g = wpool.tile([128, 4], F32, tag="gvec", name="gvec")
            cursor = 0
            for li, (cin, cout) in enumerate(CHANNELS):
                fm_tiles, chw = feats[li]
                for hh, fm in enumerate(fm_tiles):
                    done = 0
                    while done < chw:
                        take = min(128 - (cursor % 128), chw - done)
                        chunk, rowoff = cursor // 128, cursor % 128
                        mx = wpool.tile([128, 1], F32, tag="gmax",
                                        name=f"gmx{cursor}")
                        mn = wpool.tile([128, 1], F32, tag="gmean",
                                        name=f"gmn{cursor}")
                        nc.vector.tensor_reduce(mx[rowoff:rowoff + take, :],
                                                fm[done:done + take, :],
                                                axis=AX.X, op=Alu.max)
                        nc.vector.tensor_reduce(mn[rowoff:rowoff + take, :],
                                                fm[done:done + take, :],
                                                axis=AX.X, op=Alu.add)
                        nc.vector.tensor_scalar(mn[rowoff:rowoff + take, :],
                                                mn[rowoff:rowoff + take, :],
                                                1.0 / N, None, op0=Alu.mult)
                        nc.vector.tensor_tensor(g[rowoff:rowoff + take,
                                                  chunk:chunk + 1],
                                                mx[rowoff:rowoff + take, :],
                                                mn[rowoff:rowoff + take, :],
                                                op=Alu.add)
                        cursor += take
                        done += take

            # ---- head ----
            def vec_mm(vin, wname, in_dim, out_dim, bname, relu):
                nin = (in_dim + 127) // 128
                nout = (out_dim + 127) // 128
                vout = wpool.tile([128, nout], F32, tag=f"v_{wname}",
                                  name=f"v_{wname}")
                if bname is not None:
                    bt = wpool.tile([128, nout], F32, tag="headb",
                                    name=f"b_{wname}")
                    nc.sync.dma_start(bt[:, :nout], dram[bname][:, :nout])
                for oc in range(nout):
                    ow = min(128, out_dim - oc * 128)
                    ps = psum([128, 1])
                    for ic in range(nin):
                        iw = min(128, in_dim - ic * 128)
                        wt = wpool.tile([128, 1024], F32, tag="headw", bufs=3,
                                        name=f"w_{wname}_{oc}_{ic}")
                        nc.sync.dma_start(
                            wt[:iw, :ow],
                            dram[f"{wname}_{ic}"][:iw, oc * 128:oc * 128 + ow])
                        nc.tensor.matmul(
                            ps[:ow, :], wt[:iw, :ow],
                            vin[:iw, ic:ic + 1],
                            start=(ic == 0), stop=(ic == nin - 1))
                    if bname is not None:
                        nc.vector.tensor_tensor(
                            ps[:ow, :], ps[:ow, :],
                            bt[:ow, oc:oc + 1], op=Alu.add)
                    nc.scalar.activation(vout[:ow, oc:oc + 1], ps[:ow, :],
                                         Act.Relu if relu else Act.Copy)
                return vout

            v = vec_mm(g, "hw_v", 512, 512, None, False)
            v = vec_mm(v, "hw_proj", 512, 512, "hb_proj", False)
            v = vec_mm(v, "hw_pf1", 512, 1024, "hb_pf1", True)
            v = vec_mm(v, "hw_pf2", 1024, 512, "hb_pf2", True)
            v = vec_mm(v, "hw_o1", 512, 256, "hb_o1", True)
            v = vec_mm(v, "hw_o2", 256, 128, "hb_o2", True)
            v = vec_mm(v, "hw_o3", 128, 49, "hb_o3", False)
            outt = wpool.tile([NUM_CLASSES, 1], F32, tag="outt", name="outt")
            nc.vector.tensor_copy(outt[:], v[:NUM_CLASSES, :1])
            nc.sync.dma_start(out_d[:], outt[:])
    return nc


# --------------------------------------------------------------------------
# Public entry
# --------------------------------------------------------------------------
def _aux_shapes(aux):
    import ml_dtypes
    shp = {}
    for name, arr in aux.items():
        dt = {np.dtype(np.float32): "float32",
              np.dtype(ml_dtypes.bfloat16): "bfloat16",
              np.dtype(np.float16): "float16",
              np.dtype(np.uint32): "uint32"}[arr.dtype]
        shp[name] = (arr.shape, dt)
    return shp


def build_nc(aux, num_devices=8, debug=False):
    import concourse.bacc as bacc
    nc = bacc.Bacc("TRN2", target_bir_lowering=False, debug=False,
                   enable_asserts=True, num_devices=num_devices)
    _build(nc, _aux_shapes(aux), debug=debug)
    nc.compile()
    return nc


def kernel(data, params):
    """data: [8, 1024, 3] fp32; params: pytree dict. Returns [8, 49] fp32."""
    from concourse import bass_utils

    data = np.asarray(data)
    aux = _prep_host_inputs(params)
    if "nc" not in _CACHE:
        _CACHE["nc"] = build_nc(aux, num_devices=8)
    nc = _CACHE["nc"]

    in_maps = []
    for c in range(8):
        m = dict(aux)
        m["data"] = np.ascontiguousarray(data[c].T.astype(np.float32))
        in_maps.append(m)
    res = bass_utils.run_bass_kernel_spmd(nc, in_maps, core_ids=list(range(8)))
    out = np.stack([res.results[c]["out"][:, 0] for c in range(8)], axis=0)
    return out.astype(np.float32)
